# revision 41
# baseline (speedup 1.0000x reference)
"""Trainium2 Bass kernel for MultiHeadFAVORAttention (Performer, causal).

Sharding: 8 cores = 4 batches x 2 head-groups (4 heads each).
Algorithm: chunked linear attention (chunk C=128) -- the causal scan over
L=2048 becomes per-chunk matmuls:
  A~[j,i]   = sum_m kp[j,m] qp[i,m]          (masked j<=i, intra-chunk)
  num'[i]   = maskedA~.T @ V' + QP.T @ S'    (V' has a ones column -> den)
  S'       += KP.T @ V'                      (PSUM-resident running state)
  attn      = num/den; out = attnT.T @ Wo    (partial; host sums head-groups)

All matmuls are K=128 / M=128 via head-pair packing and block-diagonal
projection weights. Host pre-transposes activations/weights so no on-device
input transposes are needed. Constants ship as two packed tensors (one DMA
each); activations as one strided DMA per tensor.
"""
import math
import os

import numpy as np

import concourse.bass as bass
import concourse.mybir as mybir
import concourse.tile as tile
from concourse import bacc, bass_utils

# ---------------------------------------------------------------- constants
B, L, DIN = 4, 2048, 512
HEADS, D, M = 8, 64, 256
NH = 4            # heads per core
C = 128           # scan chunk
NCH = L // C      # 16 chunks
NW = 4            # chunks per feature window (window = 512 cols)
STAB = 1e-5
RATIO = 1.0 / math.sqrt(M)
N_CORES = 8

_F32 = mybir.dt.float32
_DT_NAME = os.environ.get("KERNEL_MM_DT", "bfloat16")
_MM_DT = getattr(mybir.dt, _DT_NAME)
_MM_NP = {"bfloat16": "bfloat16", "float32": "float32"}[_DT_NAME]

# packed const_dt column offsets (elements)
_OFF_WQ, _OFF_WK, _OFF_WV = 0, 1024, 2048
_OFF_PRJE, _OFF_PRJO = 3072, 3328
_OFF_WO = 3584
_OFF_ID = 4608
_W_CDT = 4736
# const_f32: mask 0:128, qb 128:130, kb 130:132
_W_CF = 132

_CACHED = {}


def _build_nc():
    """Build the SPMD Bass program (identical on all 8 cores)."""
    nc = bacc.Bacc("TRN2", target_bir_lowering=False, debug=False,
                   num_devices=N_CORES)
    DT = _MM_DT

    xqT = nc.dram_tensor("xqT", [DIN, L], DT, kind="ExternalInput").ap()
    xkT = nc.dram_tensor("xkT", [DIN, L], DT, kind="ExternalInput").ap()
    xvT = nc.dram_tensor("xvT", [DIN, L], DT, kind="ExternalInput").ap()
    cdt = nc.dram_tensor("cdt", [128, _W_CDT], DT, kind="ExternalInput").ap()
    cf32 = nc.dram_tensor("cf32", [128, _W_CF], _F32, kind="ExternalInput").ap()
    outp = nc.dram_tensor("outp", [L, 512], _F32, kind="ExternalOutput").ap()

    ACT = mybir.ActivationFunctionType
    ALU = mybir.AluOpType

    with tile.TileContext(nc) as tc:
        with (
            tc.tile_pool(name="const", bufs=1) as const,
            tc.tile_pool(name="xp", bufs=1) as xp,
            tc.tile_pool(name="qk", bufs=1) as qk,
            tc.tile_pool(name="vp", bufs=1) as vpool,
            tc.tile_pool(name="featq", bufs=2) as featq,
            tc.tile_pool(name="featk", bufs=2) as featk,
            tc.tile_pool(name="kpp", bufs=6) as kpp,
            tc.tile_pool(name="small", bufs=6) as small,
            tc.tile_pool(name="att", bufs=1) as att,
            tc.tile_pool(name="outs", bufs=3) as outs,
            tc.tile_pool(name="psA", bufs=3, space="PSUM") as psA,
            tc.tile_pool(name="psBig", bufs=3, space="PSUM") as psBig,
            tc.tile_pool(name="psS", bufs=1, space="PSUM") as psS,
        ):
            # ---------------- packed constants first (3 DMAs: the wq/wk
            # block lands first so the first QT matmuls start early)
            cdt_sb = const.tile([128, _W_CDT], DT)
            nc.sync.dma_start(cdt_sb[:, 0:2048], cdt[:, 0:2048])
            cf_sb = const.tile([128, _W_CF], _F32)
            nc.sync.dma_start(cf_sb[:], cf32[:])
            stab_sb = const.tile([128, 1], _F32)
            nc.vector.memset(stab_sb[:], STAB)

            # ------------- input activations: quarter DMAs interleaved
            # across the three tensors so window-0 data lands first
            xq_sb = xp.tile([128, 4, L], DT, tag="xq")
            xk_sb = xp.tile([128, 4, L], DT, tag="xk")
            xv_sb = xp.tile([128, 4, L], DT, tag="xv")
            for nt in range(4):
                for x_sb, xT in ((xq_sb, xqT), (xk_sb, xkT), (xv_sb, xvT)):
                    src = xT.rearrange("(ko p) l -> p ko l", p=128)
                    nc.sync.dma_start(x_sb[:, :, nt * 512:(nt + 1) * 512],
                                      src[:, :, nt * 512:(nt + 1) * 512])
                if nt == 0:
                    nc.sync.dma_start(cdt_sb[:, 2048:], cdt[:, 2048:])

            wq_sb = cdt_sb[:, _OFF_WQ:_OFF_WQ + 1024].rearrange(
                "p (ko x) -> p ko x", ko=4)
            wk_sb = cdt_sb[:, _OFF_WK:_OFF_WK + 1024].rearrange(
                "p (ko x) -> p ko x", ko=4)
            wv_sb = cdt_sb[:, _OFF_WV:_OFF_WV + 1024].rearrange(
                "p (ko x) -> p ko x", ko=4)
            prjE_sb = cdt_sb[:, _OFF_PRJE:_OFF_PRJE + 256]
            prjO_sb = cdt_sb[:, _OFF_PRJO:_OFF_PRJO + 256]
            prjEO_sb = cdt_sb[:, _OFF_PRJE:_OFF_PRJE + 512]
            wo_sb = cdt_sb[:, _OFF_WO:_OFF_WO + 1024].rearrange(
                "p (mh x) -> p mh x", mh=2)
            id_sb = cdt_sb[:, _OFF_ID:_OFF_ID + 128]
            mask_sb = cf_sb[:, 0:128]
            qb_sb = cf_sb[:, 128:130]
            kb_sb = cf_sb[:, 130:132]

            # ---------------- QT / KT projections, streamed per window
            QT_sb = qk.tile([128, 2, L], DT)
            KT_sb = qk.tile([128, 2, L], DT)

            def emit_qkt_part(nt, part):
                # part 0..3 -> (tensor, mt)
                x_sb, w_sb, dst, b_sb = (
                    (xq_sb, wq_sb, QT_sb, qb_sb),
                    (xk_sb, wk_sb, KT_sb, kb_sb))[part // 2]
                mt = part % 2
                ps = psBig.tile([128, 512], _F32, tag="big")
                for ko in range(4):
                    nc.tensor.matmul(
                        ps[:],
                        w_sb[:, ko, mt * 128:(mt + 1) * 128],
                        x_sb[:, ko, nt * 512:(nt + 1) * 512],
                        start=(ko == 0), stop=(ko == 3))
                if mt == 0:
                    nc.scalar.activation(
                        dst[:, mt, nt * 512:(nt + 1) * 512], ps[:],
                        ACT.Identity, bias=b_sb[:, mt:mt + 1])
                else:
                    nc.vector.tensor_scalar(
                        dst[:, mt, nt * 512:(nt + 1) * 512], ps[:],
                        b_sb[:, mt:mt + 1], None, ALU.add)

            def emit_qkt_window(nt):
                for part in range(4):
                    emit_qkt_part(nt, part)

            # ---------------- V projection -> Vp [128, NCH, 4*66] (+ones col)
            Vp = vpool.tile([128, NCH, 4 * 66], DT)
            nc.gpsimd.memset(Vp[:, :, 64::66], 1.0)

            def emit_v(lt):
                ps = psBig.tile([128, 512], _F32, tag="big")
                for ko in range(4):
                    nc.tensor.matmul(
                        ps[:, :256],
                        xv_sb[:, ko, lt * 128:(lt + 1) * 128],
                        wv_sb[:, ko, :],
                        start=(ko == 0), stop=(ko == 3))
                vdst = Vp[:, lt, :].rearrange("p (h x) -> p h x", h=4)[:, :, 0:64]
                vsrc = ps[:, 0:256].rearrange("p (h x) -> p h x", h=4)
                nc.vector.tensor_copy(vdst, vsrc)


            # ---------------- state PSUM (persistent) + SBUF shadow
            S_ps = [psS.tile([128, 264], _F32, tag=f"S{mh}", name=f"S{mh}")
                    for mh in range(2)]
            S_sb = const.tile([128, 2, 264], DT)

            attnT = att.tile([128, 2, L], DT)

            _wins = {}

            def alloc_win(w):
                QPw = featq.tile([128, 4, 2, 512], DT, tag="qw", name=f"qw{w}")
                KPw = featk.tile([128, 4, 2, 512], DT, tag="kw", name=f"kw{w}")
                _wins[w] = (QPw, KPw)

            def emit_feature_part(w, part):
                """part = (mt, hh) index 0..3; emits 4 MMs + 4 relus."""
                QPw, KPw = _wins[w]
                lo, hi = w * 512, (w + 1) * 512
                mt, hh = divmod(part, 2)
                h = 2 * mt + hh
                prj = prjE_sb if hh == 0 else prjO_sb
                for mh in range(2):
                    psq = psBig.tile([128, 512], _F32, tag="big")
                    nc.tensor.matmul(
                        psq[:], prj[:, mh * 128:(mh + 1) * 128],
                        QT_sb[:, mt, lo:hi], start=True, stop=True)
                    nc.scalar.activation(
                        QPw[:, h, mh, :], psq[:], ACT.Relu, bias=stab_sb[:])
                    psk = psBig.tile([128, 512], _F32, tag="big")
                    nc.tensor.matmul(
                        psk[:], prj[:, mh * 128:(mh + 1) * 128],
                        KT_sb[:, mt, lo:hi], start=True, stop=True)
                    if mh == 0:
                        nc.vector.tensor_scalar(
                            KPw[:, h, mh, :], psk[:], STAB, None, ALU.max)
                    else:
                        nc.scalar.activation(
                            KPw[:, h, mh, :], psk[:], ACT.Relu, bias=stab_sb[:])

            # prologue: all projections up front (PE-efficient phase),
            # only feature windows stream into the scan
            for nt in range(4):
                emit_qkt_window(nt)
            for lt in range(NCH):
                emit_v(lt)
            alloc_win(0)
            for part in range(4):
                emit_feature_part(0, part)

            # ---------------- scan (next window's features pipelined in;
            # transpose/out-proj of chunk c-1 overlapped into chunk c)
            pending = None  # (c_prev, attn4_prev)

            def emit_tail(prev_c, prev_attn4, tail):
                # transposes + attnT evacuation for a finished chunk
                pst = psA.tile([128, 256], DT, tag="A", name="pst")
                for mt in range(2):
                    nc.tensor.transpose(
                        pst[:, mt * 128:(mt + 1) * 128],
                        prev_attn4[:, mt * 128:(mt + 1) * 128], id_sb)
                nc.vector.tensor_copy(
                    attnT[:, :, prev_c * 128:(prev_c + 1) * 128],
                    pst[:].rearrange("p (mh x) -> p mh x", mh=2))
                if tail:
                    emit_outproj(prev_c)

            def emit_outproj(prev_c):
                pso = psBig.tile([128, 512], _F32, tag="big")
                for mh in range(2):
                    nc.tensor.matmul(
                        pso[:],
                        attnT[:, mh, prev_c * 128:(prev_c + 1) * 128],
                        wo_sb[:, mh, :],
                        start=(mh == 0), stop=(mh == 1))
                o_sb = outs.tile([128, 512], _F32, tag="o")
                nc.scalar.activation(o_sb[:], pso[:], ACT.Copy)
                nc.sync.dma_start(outp[prev_c * 128:(prev_c + 1) * 128, :],
                                  o_sb[:])

            for w in range(NW):
                QPw, KPw = _wins[w]
                for cc in range(NW):
                    c = w * NW + cc
                    cl, ch = cc * 128, (cc + 1) * 128

                    # KP natural chunks: one N=512 MM per pair against the
                    # contiguous [prjE | prjO] block
                    kp_mt = []
                    for mt in range(2):
                        pskp = psBig.tile([128, 512], _F32, tag="big")
                        nc.tensor.matmul(
                            pskp[:],
                            KT_sb[:, mt, c * 128:(c + 1) * 128],
                            prjEO_sb, start=True, stop=True)
                        kp2 = kpp.tile([128, 512], DT, tag="kp")
                        if mt == 0:
                            nc.vector.tensor_scalar(
                                kp2[:], pskp[:], STAB, None, ALU.max)
                        else:
                            nc.scalar.activation(
                                kp2[:], pskp[:], ACT.Relu, bias=stab_sb[:])
                        kp_mt.append(kp2)

                    # A~ for ALL 4 heads in one bank [128, 512]
                    psa = psA.tile([128, 512], _F32, tag="A", name="psa")
                    for h in range(4):
                        for mh in range(2):
                            nc.tensor.matmul(
                                psa[:, h * 128:(h + 1) * 128],
                                KPw[:, h, mh, cl:ch],
                                QPw[:, h, mh, cl:ch],
                                start=(h == 0 and mh == 0),
                                stop=(h == 3 and mh == 1),
                                skip_group_check=True)
                    mA4 = small.tile([128, 512], DT, tag="mA")
                    nc.vector.tensor_tensor(
                        mA4.rearrange("p (h x) -> p h x", h=4),
                        psa[:].rearrange("p (h x) -> p h x", h=4),
                        mask_sb[:, None, :].to_broadcast([128, 4, 128]),
                        ALU.mult)

                    # previous chunk's transposes fill the mask-wait bubble
                    if pending is not None:
                        emit_tail(*pending, tail=False)

                    # num4 [128, 264]: inter first (inputs ready), then intra
                    num4 = psA.tile([128, 264], _F32, tag="A", name="num4")
                    if c > 0:
                        for h in range(4):
                            for mh in range(2):
                                nc.tensor.matmul(
                                    num4[:, h * 66:h * 66 + 65],
                                    QPw[:, h, mh, cl:ch],
                                    S_sb[:, mh, h * 66:h * 66 + 65],
                                    start=(h == 0 and mh == 0), stop=False,
                                    skip_group_check=True)
                    for h in range(4):
                        nc.tensor.matmul(
                            num4[:, h * 66:h * 66 + 65],
                            mA4[:, h * 128:(h + 1) * 128],
                            Vp[:, c, h * 66:h * 66 + 65],
                            start=(c == 0 and h == 0), stop=(h == 3),
                            skip_group_check=True)
                    # dS += KP.T @ V'  (bank-persistent accumulation)
                    for h in range(4):
                        mt, hh = divmod(h, 2)
                        for mh in range(2):
                            nc.tensor.matmul(
                                S_ps[mh][:, h * 66:h * 66 + 65],
                                kp_mt[mt][:, hh * 256 + mh * 128:
                                          hh * 256 + (mh + 1) * 128],
                                Vp[:, c, h * 66:h * 66 + 65],
                                start=(c == 0 and h == 0),
                                stop=(c == NCH - 1 and h == 3),
                                skip_group_check=True)

                    # divide all heads at once: attn4 = num/den
                    rd4 = small.tile([128, 4], _F32, tag="rd")
                    nc.vector.reciprocal(rd4[:], num4[:, 64::66])
                    attn4 = small.tile([128, 256], DT, tag="attn2")
                    nc.vector.tensor_tensor(
                        attn4.rearrange("p (h x) -> p h x", h=4),
                        num4[:].rearrange("p (h x) -> p h x", h=4)[:, :, 0:64],
                        rd4[:, :, None].to_broadcast([128, 4, 64]),
                        ALU.mult)

                    # state copy for next chunk's inter matmuls
                    nc.vector.tensor_copy(S_sb[:, 0, :], S_ps[0][:])
                    nc.scalar.activation(S_sb[:, 1, :], S_ps[1][:], ACT.Copy)

                    # streamed feature windows fill PE idle after the
                    # chain-critical matmuls
                    if w + 1 < NW:
                        if cc == 0:
                            alloc_win(w + 1)
                        emit_feature_part(w + 1, cc)

                    # previous chunk's out-projection
                    if pending is not None:
                        emit_outproj(pending[0])
                    pending = (c, attn4)

            # flush the last chunk
            emit_tail(*pending, tail=True)

    nc.compile()
    return nc


def _host_prep(inputs):
    """Build per-core in_maps from full inputs."""
    query = np.asarray(inputs["query"], np.float32)
    key = np.asarray(inputs["key"], np.float32)
    value = np.asarray(inputs["value"], np.float32)
    proj = np.asarray(inputs["proj"], np.float32)
    w_q_w = np.asarray(inputs["w_q_w"], np.float32)
    w_q_b = np.asarray(inputs["w_q_b"], np.float32)
    w_k_w = np.asarray(inputs["w_k_w"], np.float32)
    w_k_b = np.asarray(inputs["w_k_b"], np.float32)
    w_v_w = np.asarray(inputs["w_v_w"], np.float32)
    w_o_w = np.asarray(inputs["w_o_w"], np.float32)

    mm = np.dtype(_MM_NP)
    in_maps = []
    for core in range(N_CORES):
        b, hg = divmod(core, 2)
        hsl = slice(hg * 256, (hg + 1) * 256)

        cdt = np.zeros((128, _W_CDT), np.float32)
        for off, wmat in ((_OFF_WQ, w_q_w), (_OFF_WK, w_k_w),
                          (_OFF_WV, w_v_w)):
            wT = wmat[hsl].T  # [512, 256]
            for ko in range(4):
                cdt[:, off + ko * 256:off + (ko + 1) * 256] = \
                    wT[ko * 128:(ko + 1) * 128]
        cdt[0:64, _OFF_PRJE:_OFF_PRJE + 256] = proj.T * RATIO
        cdt[64:128, _OFF_PRJO:_OFF_PRJO + 256] = proj.T * RATIO
        woT = w_o_w[:, hsl].T  # [256, 512]
        for mh in range(2):
            cdt[:, _OFF_WO + mh * 512:_OFF_WO + (mh + 1) * 512] = \
                woT[mh * 128:(mh + 1) * 128]
        cdt[:, _OFF_ID:_OFF_ID + 128] = np.eye(128, dtype=np.float32)

        cf = np.zeros((128, _W_CF), np.float32)
        cf[:, 0:128] = np.triu(np.ones((128, 128), np.float32))
        cf[:, 128:130] = w_q_b[hsl].reshape(2, 128).T
        cf[:, 130:132] = w_k_b[hsl].reshape(2, 128).T

        m = {
            "xqT": np.ascontiguousarray(query[b].T).astype(mm),
            "xkT": np.ascontiguousarray(key[b].T).astype(mm),
            "xvT": np.ascontiguousarray(value[b].T).astype(mm),
            "cdt": cdt.astype(mm),
            "cf32": cf,
        }
        in_maps.append(m)
    return in_maps


def kernel(**inputs):
    if "nc" not in _CACHED:
        _CACHED["nc"] = _build_nc()
    nc = _CACHED["nc"]

    in_maps = _host_prep(inputs)
    res = bass_utils.run_bass_kernel_spmd(
        nc, in_maps, core_ids=list(range(N_CORES)))

    w_v_b = np.asarray(inputs["w_v_b"], np.float32)
    w_o_w = np.asarray(inputs["w_o_w"], np.float32)
    w_o_b = np.asarray(inputs["w_o_b"], np.float32)

    out = np.zeros((B, L, DIN), np.float32)
    for core in range(N_CORES):
        b, hg = divmod(core, 2)
        out[b] += res.results[core]["outp"]
    # v-bias enters attn additively per dh slice: out += vb @ WoT (+ out bias)
    out += (w_v_b[None, :] @ w_o_w.T)[0][None, None, :]
    out += w_o_b[None, None, :]
    return out


# revision 43
# speedup vs baseline: 1.0058x; 1.0058x over previous
"""Trainium2 Bass kernel for MultiHeadFAVORAttention (Performer, causal).

Sharding: 8 cores = 4 batches x 2 head-groups (4 heads each).
Algorithm: chunked linear attention (chunk C=128) -- the causal scan over
L=2048 becomes per-chunk matmuls:
  A~[j,i]   = sum_m kp[j,m] qp[i,m]          (masked j<=i, intra-chunk)
  num'[i]   = maskedA~.T @ V' + QP.T @ S'    (V' has a ones column -> den)
  S'       += KP.T @ V'                      (PSUM-resident running state)
  attn      = num/den; out = attnT.T @ Wo    (partial; host sums head-groups)

All matmuls are K=128 / M=128 via head-pair packing and block-diagonal
projection weights. Host pre-transposes activations/weights so no on-device
input transposes are needed. Constants ship as two packed tensors (one DMA
each); activations as one strided DMA per tensor.
"""
import math
import os

import numpy as np

import concourse.bass as bass
import concourse.mybir as mybir
import concourse.tile as tile
from concourse import bacc, bass_utils

# ---------------------------------------------------------------- constants
B, L, DIN = 4, 2048, 512
HEADS, D, M = 8, 64, 256
NH = 4            # heads per core
C = 128           # scan chunk
NCH = L // C      # 16 chunks
NW = 4            # chunks per feature window (window = 512 cols)
STAB = 1e-5
RATIO = 1.0 / math.sqrt(M)
N_CORES = 8

_F32 = mybir.dt.float32
_DT_NAME = os.environ.get("KERNEL_MM_DT", "bfloat16")
_MM_DT = getattr(mybir.dt, _DT_NAME)
_MM_NP = {"bfloat16": "bfloat16", "float32": "float32"}[_DT_NAME]

# packed const_dt column offsets (elements)
_OFF_WQ, _OFF_WK, _OFF_WV = 0, 1024, 2048
_OFF_PRJE, _OFF_PRJO = 3072, 3328
_OFF_WO = 3584
_OFF_ID = 4608
_W_CDT = 4736
# const_f32: mask 0:128, qb 128:130, kb 130:132
_W_CF = 132

_CACHED = {}


def _build_nc():
    """Build the SPMD Bass program (identical on all 8 cores)."""
    nc = bacc.Bacc("TRN2", target_bir_lowering=False, debug=False,
                   num_devices=N_CORES)
    DT = _MM_DT

    xqT = nc.dram_tensor("xqT", [DIN, L], DT, kind="ExternalInput").ap()
    xkT = nc.dram_tensor("xkT", [DIN, L], DT, kind="ExternalInput").ap()
    xvT = nc.dram_tensor("xvT", [DIN, L], DT, kind="ExternalInput").ap()
    cdt = nc.dram_tensor("cdt", [128, _W_CDT], DT, kind="ExternalInput").ap()
    cf32 = nc.dram_tensor("cf32", [128, _W_CF], _F32, kind="ExternalInput").ap()
    outp = nc.dram_tensor("outp", [L, 512], _F32, kind="ExternalOutput").ap()

    ACT = mybir.ActivationFunctionType
    ALU = mybir.AluOpType

    with tile.TileContext(nc) as tc:
        with (
            tc.tile_pool(name="const", bufs=1) as const,
            tc.tile_pool(name="xp", bufs=1) as xp,
            tc.tile_pool(name="qk", bufs=1) as qk,
            tc.tile_pool(name="vp", bufs=1) as vpool,
            tc.tile_pool(name="featq", bufs=2) as featq,
            tc.tile_pool(name="featk", bufs=2) as featk,
            tc.tile_pool(name="kpp", bufs=6) as kpp,
            tc.tile_pool(name="small", bufs=6) as small,
            tc.tile_pool(name="att", bufs=1) as att,
            tc.tile_pool(name="outs", bufs=3) as outs,
            tc.tile_pool(name="psA", bufs=3, space="PSUM") as psA,
            tc.tile_pool(name="psBig", bufs=3, space="PSUM") as psBig,
            tc.tile_pool(name="psS", bufs=1, space="PSUM") as psS,
        ):
            # ---------------- packed constants first (3 DMAs: the wq/wk
            # block lands first so the first QT matmuls start early)
            cdt_sb = const.tile([128, _W_CDT], DT)
            nc.sync.dma_start(cdt_sb[:, 0:2048], cdt[:, 0:2048])
            cf_sb = const.tile([128, _W_CF], _F32)
            nc.sync.dma_start(cf_sb[:], cf32[:])
            stab_sb = const.tile([128, 1], _F32)
            nc.vector.memset(stab_sb[:], STAB)

            # ------------- input activations: quarter DMAs interleaved
            # across the three tensors so window-0 data lands first
            xq_sb = xp.tile([128, 4, L], DT, tag="xq")
            xk_sb = xp.tile([128, 4, L], DT, tag="xk")
            xv_sb = xp.tile([128, 4, L], DT, tag="xv")
            for nt in range(4):
                for x_sb, xT in ((xq_sb, xqT), (xk_sb, xkT), (xv_sb, xvT)):
                    src = xT.rearrange("(ko p) l -> p ko l", p=128)
                    nc.sync.dma_start(x_sb[:, :, nt * 512:(nt + 1) * 512],
                                      src[:, :, nt * 512:(nt + 1) * 512])
                if nt == 0:
                    nc.sync.dma_start(cdt_sb[:, 2048:], cdt[:, 2048:])

            wq_sb = cdt_sb[:, _OFF_WQ:_OFF_WQ + 1024].rearrange(
                "p (ko x) -> p ko x", ko=4)
            wk_sb = cdt_sb[:, _OFF_WK:_OFF_WK + 1024].rearrange(
                "p (ko x) -> p ko x", ko=4)
            wv_sb = cdt_sb[:, _OFF_WV:_OFF_WV + 1024].rearrange(
                "p (ko x) -> p ko x", ko=4)
            prjE_sb = cdt_sb[:, _OFF_PRJE:_OFF_PRJE + 256]
            prjO_sb = cdt_sb[:, _OFF_PRJO:_OFF_PRJO + 256]
            prjEO_sb = cdt_sb[:, _OFF_PRJE:_OFF_PRJE + 512]
            wo_sb = cdt_sb[:, _OFF_WO:_OFF_WO + 1024].rearrange(
                "p (mh x) -> p mh x", mh=2)
            id_sb = cdt_sb[:, _OFF_ID:_OFF_ID + 128]
            mask_sb = cf_sb[:, 0:128]
            qb_sb = cf_sb[:, 128:130]
            kb_sb = cf_sb[:, 130:132]

            # ---------------- QT / KT projections, streamed per window
            QT_sb = qk.tile([128, 2, L], DT)
            KT_sb = qk.tile([128, 2, L], DT)

            def emit_qkt_part(nt, part):
                # part 0..3 -> (tensor, mt)
                x_sb, w_sb, dst, b_sb = (
                    (xq_sb, wq_sb, QT_sb, qb_sb),
                    (xk_sb, wk_sb, KT_sb, kb_sb))[part // 2]
                mt = part % 2
                ps = psBig.tile([128, 512], _F32, tag="big")
                for ko in range(4):
                    nc.tensor.matmul(
                        ps[:],
                        w_sb[:, ko, mt * 128:(mt + 1) * 128],
                        x_sb[:, ko, nt * 512:(nt + 1) * 512],
                        start=(ko == 0), stop=(ko == 3))
                if mt == 0:
                    nc.scalar.activation(
                        dst[:, mt, nt * 512:(nt + 1) * 512], ps[:],
                        ACT.Identity, bias=b_sb[:, mt:mt + 1])
                else:
                    nc.vector.tensor_scalar(
                        dst[:, mt, nt * 512:(nt + 1) * 512], ps[:],
                        b_sb[:, mt:mt + 1], None, ALU.add)

            def emit_qkt_window(nt):
                for part in range(4):
                    emit_qkt_part(nt, part)

            # ---------------- V projection -> Vp [128, NCH, 4*66] (+ones col)
            Vp = vpool.tile([128, NCH, 4 * 66], DT)
            nc.gpsimd.memset(Vp[:, :, 64::66], 1.0)

            def emit_v(lt):
                ps = psBig.tile([128, 512], _F32, tag="big")
                for ko in range(4):
                    nc.tensor.matmul(
                        ps[:, :256],
                        xv_sb[:, ko, lt * 128:(lt + 1) * 128],
                        wv_sb[:, ko, :],
                        start=(ko == 0), stop=(ko == 3))
                vdst = Vp[:, lt, :].rearrange("p (h x) -> p h x", h=4)[:, :, 0:64]
                vsrc = ps[:, 0:256].rearrange("p (h x) -> p h x", h=4)
                nc.vector.tensor_copy(vdst, vsrc)


            # ---------------- state PSUM (persistent) + SBUF shadow
            S_ps = [psS.tile([128, 264], _F32, tag=f"S{mh}", name=f"S{mh}")
                    for mh in range(2)]
            S_sb = const.tile([128, 2, 264], DT)

            attnT = att.tile([128, 2, L], DT)

            _wins = {}

            def alloc_win(w):
                QPw = featq.tile([128, 4, 2, 512], DT, tag="qw", name=f"qw{w}")
                KPw = featk.tile([128, 4, 2, 512], DT, tag="kw", name=f"kw{w}")
                _wins[w] = (QPw, KPw)

            def emit_feature_part(w, part):
                """part = (mt, hh) index 0..3; emits 4 MMs + 4 relus."""
                QPw, KPw = _wins[w]
                lo, hi = w * 512, (w + 1) * 512
                mt, hh = divmod(part, 2)
                h = 2 * mt + hh
                prj = prjE_sb if hh == 0 else prjO_sb
                for mh in range(2):
                    psq = psBig.tile([128, 512], _F32, tag="big")
                    nc.tensor.matmul(
                        psq[:], prj[:, mh * 128:(mh + 1) * 128],
                        QT_sb[:, mt, lo:hi], start=True, stop=True)
                    nc.scalar.activation(
                        QPw[:, h, mh, :], psq[:], ACT.Relu, bias=stab_sb[:])
                    psk = psBig.tile([128, 512], _F32, tag="big")
                    nc.tensor.matmul(
                        psk[:], prj[:, mh * 128:(mh + 1) * 128],
                        KT_sb[:, mt, lo:hi], start=True, stop=True)
                    if mh == 0:
                        nc.vector.tensor_scalar(
                            KPw[:, h, mh, :], psk[:], STAB, None, ALU.max)
                    else:
                        nc.scalar.activation(
                            KPw[:, h, mh, :], psk[:], ACT.Relu, bias=stab_sb[:])

            # prologue: all projections up front (PE-efficient phase),
            # only feature windows stream into the scan
            for nt in range(4):
                emit_qkt_window(nt)
            for lt in range(NCH):
                emit_v(lt)
            kp_store = {}

            def emit_kp(kc):
                # KP natural chunk: one N=512 MM per pair against the
                # contiguous [prjE | prjO] block; emitted one chunk ahead
                kp_mt = []
                for mt in range(2):
                    pskp = psBig.tile([128, 512], _F32, tag="big")
                    nc.tensor.matmul(
                        pskp[:],
                        KT_sb[:, mt, kc * 128:(kc + 1) * 128],
                        prjEO_sb, start=True, stop=True)
                    kp2 = kpp.tile([128, 512], DT, tag="kp")
                    if mt == 0:
                        nc.vector.tensor_scalar(
                            kp2[:], pskp[:], STAB, None, ALU.max)
                    else:
                        nc.scalar.activation(
                            kp2[:], pskp[:], ACT.Relu, bias=stab_sb[:])
                    kp_mt.append(kp2)
                kp_store[kc] = kp_mt

            alloc_win(0)
            for part in range(4):
                emit_feature_part(0, part)
            emit_kp(0)

            # ---------------- scan (next window's features pipelined in;
            # transpose/out-proj of chunk c-1 overlapped into chunk c)
            pending = None  # (c_prev, attn4_prev)

            def emit_tail(prev_c, prev_attn4, tail):
                # transposes + attnT evacuation for a finished chunk
                pst = psA.tile([128, 256], DT, tag="A", name="pst")
                for mt in range(2):
                    nc.tensor.transpose(
                        pst[:, mt * 128:(mt + 1) * 128],
                        prev_attn4[:, mt * 128:(mt + 1) * 128], id_sb)
                nc.vector.tensor_copy(
                    attnT[:, :, prev_c * 128:(prev_c + 1) * 128],
                    pst[:].rearrange("p (mh x) -> p mh x", mh=2))
                if tail:
                    emit_outproj(prev_c)

            def emit_outproj(prev_c):
                pso = psBig.tile([128, 512], _F32, tag="big")
                for mh in range(2):
                    nc.tensor.matmul(
                        pso[:],
                        attnT[:, mh, prev_c * 128:(prev_c + 1) * 128],
                        wo_sb[:, mh, :],
                        start=(mh == 0), stop=(mh == 1))
                o_sb = outs.tile([128, 512], _F32, tag="o")
                nc.scalar.activation(o_sb[:], pso[:], ACT.Copy)
                nc.sync.dma_start(outp[prev_c * 128:(prev_c + 1) * 128, :],
                                  o_sb[:])

            for w in range(NW):
                QPw, KPw = _wins[w]
                for cc in range(NW):
                    c = w * NW + cc
                    cl, ch = cc * 128, (cc + 1) * 128

                    kp_mt = kp_store.pop(c)

                    # A~ for ALL 4 heads in one bank [128, 512]
                    psa = psA.tile([128, 512], _F32, tag="A", name="psa")
                    for h in range(4):
                        for mh in range(2):
                            nc.tensor.matmul(
                                psa[:, h * 128:(h + 1) * 128],
                                KPw[:, h, mh, cl:ch],
                                QPw[:, h, mh, cl:ch],
                                start=(h == 0 and mh == 0),
                                stop=(h == 3 and mh == 1),
                                skip_group_check=True)
                    mA4 = small.tile([128, 512], DT, tag="mA")
                    nc.vector.tensor_tensor(
                        mA4.rearrange("p (h x) -> p h x", h=4),
                        psa[:].rearrange("p (h x) -> p h x", h=4),
                        mask_sb[:, None, :].to_broadcast([128, 4, 128]),
                        ALU.mult)

                    # previous chunk's transposes fill the mask-wait bubble
                    if pending is not None:
                        emit_tail(*pending, tail=False)

                    # num4 [128, 264]: inter first (inputs ready), then intra
                    num4 = psA.tile([128, 264], _F32, tag="A", name="num4")
                    if c > 0:
                        for h in range(4):
                            for mh in range(2):
                                nc.tensor.matmul(
                                    num4[:, h * 66:h * 66 + 65],
                                    QPw[:, h, mh, cl:ch],
                                    S_sb[:, mh, h * 66:h * 66 + 65],
                                    start=(h == 0 and mh == 0), stop=False,
                                    skip_group_check=True)
                    for h in range(4):
                        nc.tensor.matmul(
                            num4[:, h * 66:h * 66 + 65],
                            mA4[:, h * 128:(h + 1) * 128],
                            Vp[:, c, h * 66:h * 66 + 65],
                            start=(c == 0 and h == 0), stop=(h == 3),
                            skip_group_check=True)
                    # dS += KP.T @ V'  (bank-persistent accumulation)
                    for h in range(4):
                        mt, hh = divmod(h, 2)
                        for mh in range(2):
                            nc.tensor.matmul(
                                S_ps[mh][:, h * 66:h * 66 + 65],
                                kp_mt[mt][:, hh * 256 + mh * 128:
                                          hh * 256 + (mh + 1) * 128],
                                Vp[:, c, h * 66:h * 66 + 65],
                                start=(c == 0 and h == 0),
                                stop=(c == NCH - 1 and h == 3),
                                skip_group_check=True)

                    # divide all heads at once: attn4 = num/den
                    rd4 = small.tile([128, 4], _F32, tag="rd")
                    nc.vector.reciprocal(rd4[:], num4[:, 64::66])
                    attn4 = small.tile([128, 256], DT, tag="attn2")
                    nc.vector.tensor_tensor(
                        attn4.rearrange("p (h x) -> p h x", h=4),
                        num4[:].rearrange("p (h x) -> p h x", h=4)[:, :, 0:64],
                        rd4[:, :, None].to_broadcast([128, 4, 64]),
                        ALU.mult)

                    # state copy for next chunk's inter matmuls
                    nc.vector.tensor_copy(S_sb[:, 0, :], S_ps[0][:])
                    nc.scalar.activation(S_sb[:, 1, :], S_ps[1][:], ACT.Copy)

                    # streamed feature windows + next chunk's KP fill PE
                    # idle after the chain-critical matmuls
                    if c + 1 < NCH:
                        emit_kp(c + 1)
                    if w + 1 < NW:
                        if cc == 0:
                            alloc_win(w + 1)
                        emit_feature_part(w + 1, cc)

                    # previous chunk's out-projection
                    if pending is not None:
                        emit_outproj(pending[0])
                    pending = (c, attn4)

            # flush the last chunk
            emit_tail(*pending, tail=True)

    nc.compile()
    return nc


def _host_prep(inputs):
    """Build per-core in_maps from full inputs."""
    query = np.asarray(inputs["query"], np.float32)
    key = np.asarray(inputs["key"], np.float32)
    value = np.asarray(inputs["value"], np.float32)
    proj = np.asarray(inputs["proj"], np.float32)
    w_q_w = np.asarray(inputs["w_q_w"], np.float32)
    w_q_b = np.asarray(inputs["w_q_b"], np.float32)
    w_k_w = np.asarray(inputs["w_k_w"], np.float32)
    w_k_b = np.asarray(inputs["w_k_b"], np.float32)
    w_v_w = np.asarray(inputs["w_v_w"], np.float32)
    w_o_w = np.asarray(inputs["w_o_w"], np.float32)

    mm = np.dtype(_MM_NP)
    in_maps = []
    for core in range(N_CORES):
        b, hg = divmod(core, 2)
        hsl = slice(hg * 256, (hg + 1) * 256)

        cdt = np.zeros((128, _W_CDT), np.float32)
        for off, wmat in ((_OFF_WQ, w_q_w), (_OFF_WK, w_k_w),
                          (_OFF_WV, w_v_w)):
            wT = wmat[hsl].T  # [512, 256]
            for ko in range(4):
                cdt[:, off + ko * 256:off + (ko + 1) * 256] = \
                    wT[ko * 128:(ko + 1) * 128]
        cdt[0:64, _OFF_PRJE:_OFF_PRJE + 256] = proj.T * RATIO
        cdt[64:128, _OFF_PRJO:_OFF_PRJO + 256] = proj.T * RATIO
        woT = w_o_w[:, hsl].T  # [256, 512]
        for mh in range(2):
            cdt[:, _OFF_WO + mh * 512:_OFF_WO + (mh + 1) * 512] = \
                woT[mh * 128:(mh + 1) * 128]
        cdt[:, _OFF_ID:_OFF_ID + 128] = np.eye(128, dtype=np.float32)

        cf = np.zeros((128, _W_CF), np.float32)
        cf[:, 0:128] = np.triu(np.ones((128, 128), np.float32))
        cf[:, 128:130] = w_q_b[hsl].reshape(2, 128).T
        cf[:, 130:132] = w_k_b[hsl].reshape(2, 128).T

        m = {
            "xqT": np.ascontiguousarray(query[b].T).astype(mm),
            "xkT": np.ascontiguousarray(key[b].T).astype(mm),
            "xvT": np.ascontiguousarray(value[b].T).astype(mm),
            "cdt": cdt.astype(mm),
            "cf32": cf,
        }
        in_maps.append(m)
    return in_maps


def kernel(**inputs):
    if "nc" not in _CACHED:
        _CACHED["nc"] = _build_nc()
    nc = _CACHED["nc"]

    in_maps = _host_prep(inputs)
    res = bass_utils.run_bass_kernel_spmd(
        nc, in_maps, core_ids=list(range(N_CORES)))

    w_v_b = np.asarray(inputs["w_v_b"], np.float32)
    w_o_w = np.asarray(inputs["w_o_w"], np.float32)
    w_o_b = np.asarray(inputs["w_o_b"], np.float32)

    out = np.zeros((B, L, DIN), np.float32)
    for core in range(N_CORES):
        b, hg = divmod(core, 2)
        out[b] += res.results[core]["outp"]
    # v-bias enters attn additively per dh slice: out += vb @ WoT (+ out bias)
    out += (w_v_b[None, :] @ w_o_w.T)[0][None, None, :]
    out += w_o_b[None, None, :]
    return out
